# revision 10
# baseline (speedup 1.0000x reference)
"""Distributed Trainium2 Bass kernel for nn_AttentionMixture (moe_routing).

Sharding: token-data-parallel. 8192 tokens -> 1024/core; cores (2b, 2b+1)
hold batch b. Cross-core coupling (softmax over sequence axis + kc/vc token
contractions) is folded into ONE pairwise AllReduce of partial sums.
All matmuls bf16 with f32 PSUM accumulation.
"""

import math
import sys
from contextlib import ExitStack

sys.path.insert(0, "/opt/trn_rl_repo")

import numpy as np

import concourse.bass as bass
import concourse.mybir as mybir
import concourse.tile as tile
from concourse import bacc
from concourse import bass_utils

F32 = mybir.dt.float32
BF16 = mybir.dt.bfloat16
AF = mybir.ActivationFunctionType

# problem dims
G, H, DQK, DV, L, D = 8, 16, 128, 128, 100, 512
FF = D            # 512
FF2 = 2 * FF      # 1024
GH = G * H * DQK  # 16384
KV = G * (DQK + DV)  # 2048
B, S = 4, 2048
NCORES = 8
T = (B * S) // NCORES   # 1024 tokens per core
TC = T // 128           # 8 token chunks

ISQ = 1.0 / math.sqrt(DQK)
KA = 0.5 * ISQ          # coef on sum(k*w1) for kc
VA = 0.5                # coef on sum(v*w1) for vc
BR = 1.0 / L            # (b/a) ratio for the w2 term (same for k and v)


def build():
    nc = bacc.Bacc("TRN2", target_bir_lowering=False, debug=False,
                   enable_asserts=False, num_devices=NCORES)

    # ---- I/O ----
    x_d = nc.dram_tensor("x", [T, D], F32, kind="ExternalInput").ap()
    gamma_d = nc.dram_tensor("gamma_norm", [D], F32, kind="ExternalInput").ap()
    wq_d = nc.dram_tensor("Wq_w", [D, GH], F32, kind="ExternalInput").ap()
    wqb_d = nc.dram_tensor("Wq_b", [GH], F32, kind="ExternalInput").ap()
    wkv_d = nc.dram_tensor("Wkv_w", [D, KV], F32, kind="ExternalInput").ap()
    wkvb_d = nc.dram_tensor("Wkv_b", [KV], F32, kind="ExternalInput").ap()
    aw1_d = nc.dram_tensor("aw1_w", [G * DQK, L], F32, kind="ExternalInput").ap()
    aw1b_d = nc.dram_tensor("aw1_b", [L], F32, kind="ExternalInput").ap()
    aw2_d = nc.dram_tensor("aw2_w", [L, L], F32, kind="ExternalInput").ap()
    aw2b_d = nc.dram_tensor("aw2_b", [L], F32, kind="ExternalInput").ap()
    wzg1_d = nc.dram_tensor("wz_g1", [G * H * DV], F32, kind="ExternalInput").ap()
    wz1_d = nc.dram_tensor("wz_w1", [G * H * DV, FF2], F32, kind="ExternalInput").ap()
    wzb1_d = nc.dram_tensor("wz_b1", [FF2], F32, kind="ExternalInput").ap()
    wzg2_d = nc.dram_tensor("wz_g2", [FF2], F32, kind="ExternalInput").ap()
    wz2_d = nc.dram_tensor("wz_w2", [FF2, FF], F32, kind="ExternalInput").ap()
    wzb2_d = nc.dram_tensor("wz_b2", [FF], F32, kind="ExternalInput").ap()
    out_d = nc.dram_tensor("out", [T, FF], F32, kind="ExternalOutput").ap()

    with tile.TileContext(nc) as tc:
        _build_tile(nc, tc, locals())
    nc.compile()
    return nc


def _build_tile(nc, tc, io):
    x_d = io["x_d"]; gamma_d = io["gamma_d"]
    wq_d = io["wq_d"]; wqb_d = io["wqb_d"]
    wkv_d = io["wkv_d"]; wkvb_d = io["wkvb_d"]
    aw1_d = io["aw1_d"]; aw1b_d = io["aw1b_d"]
    aw2_d = io["aw2_d"]; aw2b_d = io["aw2b_d"]
    wzg1_d = io["wzg1_d"]; wz1_d = io["wz1_d"]; wzb1_d = io["wzb1_d"]
    wzg2_d = io["wzg2_d"]; wz2_d = io["wz2_d"]; wzb2_d = io["wzb2_d"]
    out_d = io["out_d"]

    from concourse.masks import make_identity

    ctx = ExitStack()
    constp = ctx.enter_context(tc.tile_pool(name="constp", bufs=1))
    pp = ctx.enter_context(tc.tile_pool(name="pp", bufs=1))
    dramp = ctx.enter_context(tc.tile_pool(name="dramp", bufs=1, space="DRAM"))

    # ---- constants ----
    idf = constp.tile([128, 128], F32)
    make_identity(nc, idf)
    idb = constp.tile([128, 128], BF16)
    nc.vector.tensor_copy(idb[:], idf[:])
    ones_c = constp.tile([1, 128], BF16)   # k=1 lhsT for bias rows / bcast
    nc.gpsimd.memset(ones_c[:], 1.0)
    ones_cf = constp.tile([1, 128], F32)
    nc.gpsimd.memset(ones_cf[:], 1.0)
    ones100 = constp.tile([100, 128], BF16)  # lhsT for r broadcast
    nc.gpsimd.memset(ones100[:], 1.0)
    ones128 = constp.tile([128, 1], BF16)    # lhsT for partition-sum
    nc.gpsimd.memset(ones128[:], 1.0)

    # ---- persistent tiles ----
    xnT = pp.tile([128, 4, T], BF16)        # x normed, transposed [d, tok]
    kcvc = pp.tile([100, 2 * G * DQK], BF16)  # [l, kc(1024) | vc(1024)]
    kcT = pp.tile([128, G, 100], BF16)      # kc transposed per group [d, l]
    h1acc = pp.tile([128, 8, FF2], F32)     # h1^T accumulator [ff2, tok]
    ssz = pp.tile([1, T], F32)              # sum of z^2 per token
    gam = pp.tile([128, 4], F32)
    aw1b = pp.tile([100, 1], F32)
    aw2b = pp.tile([100, 1], F32)
    kvb = pp.tile([128, 16], F32)           # Wkv_b for per-partition use
    kvb_r = pp.tile([1, KV], BF16)          # Wkv_b as a row (free-axis bias)
    wzb1 = pp.tile([128, 8], F32)
    wzb2_r = pp.tile([1, FF], BF16)
    g2sb = pp.tile([128, 8], F32)           # wz_g2 per-partition

    nc.sync.dma_start(gam[:], gamma_d.rearrange("(c p) -> p c", p=128))
    nc.sync.dma_start(aw1b[:], aw1b_d.rearrange("(l o) -> l o", o=1))
    nc.sync.dma_start(aw2b[:], aw2b_d.rearrange("(l o) -> l o", o=1))
    nc.sync.dma_start(kvb[:], wkvb_d.rearrange("(c p) -> p c", p=128))
    nc.sync.dma_start(wzb1[:], wzb1_d.rearrange("(c p) -> p c", p=128))
    nc.sync.dma_start(g2sb[:], wzg2_d.rearrange("(c p) -> p c", p=128))

    # ================= setup phase =================
    sctx = ExitStack()
    sp = sctx.enter_context(tc.tile_pool(name="sp", bufs=1))
    psa = sctx.enter_context(tc.tile_pool(name="psa", bufs=1, space="PSUM"))

    kvb_f = sp.tile([1, KV], F32)
    nc.sync.dma_start(kvb_f[:], wkvb_d.rearrange("(o k) -> o k", o=1))
    nc.vector.tensor_copy(kvb_r[:], kvb_f[:])
    wzb2_f = sp.tile([1, FF], F32)
    nc.sync.dma_start(wzb2_f[:], wzb2_d.rearrange("(o k) -> o k", o=1))
    nc.vector.tensor_copy(wzb2_r[:], wzb2_f[:])

    # ---- A: load x, rmsnorm, transpose -> xnT ----
    xd3 = x_d.rearrange("(n p) d -> n p d", p=128)
    for n in range(TC):
        xt = sp.tile([128, D], F32, tag="xt", bufs=2)
        nc.sync.dma_start(xt[:], xd3[n])
        sq = sp.tile([128, D], F32, tag="sq", bufs=1)
        ssx = sp.tile([128, 1], F32, tag="ssx", bufs=2)
        nc.scalar.activation(sq[:], xt[:], AF.Square, accum_out=ssx[:])
        nrm = sp.tile([128, 1], F32, tag="nrm", bufs=2)
        nc.scalar.activation(nrm[:], ssx[:], AF.Sqrt, scale=1.0 / D)
        rn = sp.tile([128, 1], F32, tag="rn", bufs=2)
        nc.vector.reciprocal(rn[:], nrm[:])
        xn = sp.tile([128, D], F32, tag="xn", bufs=2)
        nc.vector.tensor_scalar_mul(xn[:], xt[:], rn[:])
        for c in range(4):
            ptr = psa.tile([128, 128], F32, tag="ps_a", bufs=2)
            nc.tensor.transpose(ptr[:], xn[:, c * 128:(c + 1) * 128], idf[:])
            nc.scalar.activation(xnT[:, c, n * 128:(n + 1) * 128], ptr[:],
                                 AF.Identity, scale=gam[:, c:c + 1])

    # ---- B: Wkv load; k^T ; k/v token-major ----
    wkv_b = sp.tile([128, 4, KV], BF16)
    wkv3 = wkv_d.rearrange("(c p) k -> c p k", p=128)
    for c in range(4):
        wkv_f = sp.tile([128, KV], F32, tag="wkv_f", bufs=2)
        nc.sync.dma_start(wkv_f[:], wkv3[c])
        nc.vector.tensor_copy(wkv_b[:, c, :], wkv_f[:])

    kT = sp.tile([128, 8, T], BF16)   # k transposed [kfeat, tok]
    for m in range(8):
        for t in range(2):
            ps = psa.tile([128, 512], F32, tag="ps_a", bufs=2)
            for c in range(4):
                nc.tensor.matmul(ps[:], wkv_b[:, c, m * 128:(m + 1) * 128],
                                 xnT[:, c, t * 512:(t + 1) * 512],
                                 start=(c == 0), stop=(c == 3))
            nc.scalar.activation(kT[:, m, t * 512:(t + 1) * 512], ps[:],
                                 AF.Identity, bias=kvb[:, m:m + 1])

    ktok = sp.tile([128, 8, 1025], BF16)  # [tok, kfeat | ones]
    vtok = sp.tile([128, 8, 1025], BF16)  # [tok, vfeat | ones]
    for n in range(TC):
        nc.gpsimd.memset(ktok[:, n, 1024:1025], 1.0)
        nc.gpsimd.memset(vtok[:, n, 1024:1025], 1.0)
        for half in range(4):
            dst = ktok if half < 2 else vtok
            off = (half % 2) * 512
            ps = psa.tile([128, 512], F32, tag="ps_a", bufs=2)
            nc.tensor.matmul(ps[:], ones_c[:],
                             kvb_r[:, half * 512:(half + 1) * 512],
                             start=True, stop=False)
            for c in range(4):
                nc.tensor.matmul(ps[:], xnT[:, c, n * 128:(n + 1) * 128],
                                 wkv_b[:, c, half * 512:(half + 1) * 512],
                                 start=False, stop=(c == 3))
            nc.vector.tensor_copy(dst[:, n, off:off + 512], ps[:])

    # ---- C: dispatch-weight logits; w1, e ----
    aw1f = sp.tile([128, 8, L], F32)
    aw1b16 = sp.tile([128, 8, L], BF16)
    aw13 = aw1_d.rearrange("(c p) l -> c p l", p=128)
    for c in range(8):
        nc.sync.dma_start(aw1f[:, c, :], aw13[c])
        nc.vector.tensor_copy(aw1b16[:, c, :], aw1f[:, c, :])
    aw2f = sp.tile([100, L], F32)
    nc.sync.dma_start(aw2f[:], aw2_d)
    aw2b16 = sp.tile([100, L], BF16)
    nc.vector.tensor_copy(aw2b16[:], aw2f[:])

    s1T = sp.tile([100, T], BF16)
    for t in range(2):
        ps = psa.tile([100, 512], F32, tag="ps_a", bufs=2)
        for c in range(8):
            nc.tensor.matmul(ps[:], aw1b16[:, c, :],
                             kT[:, c, t * 512:(t + 1) * 512],
                             start=(c == 0), stop=(c == 7))
        nc.scalar.activation(s1T[:, t * 512:(t + 1) * 512], ps[:],
                             AF.Silu, bias=aw1b[:])
    wlT = sp.tile([100, T], F32)
    for t in range(2):
        ps = psa.tile([100, 512], F32, tag="ps_a", bufs=2)
        nc.tensor.matmul(ps[:], aw2b16[:], s1T[:, t * 512:(t + 1) * 512],
                         start=True, stop=True)
        nc.scalar.activation(wlT[:, t * 512:(t + 1) * 512], ps[:],
                             AF.Identity, bias=aw2b[:])

    w1b = sp.tile([128, 8, L], BF16)   # w1 = softmax_l / S, token-major
    eb = sp.tile([128, 8, L], BF16)    # exp(logits), token-major
    for n in range(TC):
        ptr = psa.tile([128, 128], F32, tag="ps_a", bufs=2)
        nc.tensor.transpose(ptr[:, 0:100], wlT[:, n * 128:(n + 1) * 128], idf[:100, :100])
        ef = sp.tile([128, L], F32, tag="ef", bufs=2)
        r1 = sp.tile([128, 1], F32, tag="r1", bufs=2)
        nc.scalar.activation(ef[:], ptr[:, 0:100], AF.Exp, accum_out=r1[:])
        nc.vector.tensor_copy(eb[:, n, :], ef[:])
        r1s = sp.tile([128, 1], F32, tag="r1s", bufs=2)
        nc.scalar.activation(r1s[:], r1[:], AF.Identity, scale=float(S))
        rr = sp.tile([128, 1], F32, tag="rr", bufs=2)
        nc.vector.reciprocal(rr[:], r1s[:])
        nc.vector.tensor_scalar_mul(w1b[:, n, :], ef[:], rr[:])

    # ---- D: partial contractions + AllReduce ----
    # bounce layout [100, 4101]: A1k(1024) A1v(1024) N2k(1024) N2v(1024) D2(1) pad(4)
    bin_ = dramp.tile([100, 4101], F32)
    bout = dramp.tile([100, 4101], F32)

    pk = psa.tile([100, 1024], F32, tag="pk", bufs=1)
    pv = psa.tile([100, 1024], F32, tag="pv", bufs=1)
    for n in range(TC):
        for t in range(2):
            nc.tensor.matmul(pk[:, t * 512:(t + 1) * 512], w1b[:, n, :],
                             ktok[:, n, t * 512:(t + 1) * 512],
                             start=(n == 0), stop=(n == TC - 1))
            nc.tensor.matmul(pv[:, t * 512:(t + 1) * 512], w1b[:, n, :],
                             vtok[:, n, t * 512:(t + 1) * 512],
                             start=(n == 0), stop=(n == TC - 1))
    parts = sp.tile([100, 4101], F32)
    nc.vector.tensor_copy(parts[:, 0:1024], pk[:, 0:1024])
    nc.vector.tensor_copy(parts[:, 1024:2048], pv[:, 0:1024])
    pk2 = psa.tile([100, 1024], F32, tag="pk", bufs=1)
    pv2 = psa.tile([100, 1024], F32, tag="pv", bufs=1)
    pd2 = psa.tile([100, 1], F32, tag="pd", bufs=1)
    for n in range(TC):
        for t in range(2):
            nc.tensor.matmul(pk2[:, t * 512:(t + 1) * 512], eb[:, n, :],
                             ktok[:, n, t * 512:(t + 1) * 512],
                             start=(n == 0), stop=(n == TC - 1))
            nc.tensor.matmul(pv2[:, t * 512:(t + 1) * 512], eb[:, n, :],
                             vtok[:, n, t * 512:(t + 1) * 512],
                             start=(n == 0), stop=(n == TC - 1))
        nc.tensor.matmul(pd2[:], eb[:, n, :], vtok[:, n, 1024:1025],
                         start=(n == 0), stop=(n == TC - 1))
    nc.vector.tensor_copy(parts[:, 2048:3072], pk2[:, 0:1024])
    nc.vector.tensor_copy(parts[:, 3072:4096], pv2[:, 0:1024])
    nc.vector.tensor_copy(parts[:, 4096:4097], pd2[:])
    nc.gpsimd.memset(parts[:, 4097:4101], 0.0)
    nc.sync.dma_start(bin_[:], parts[:])

    nc.gpsimd.collective_compute(
        "AllReduce", mybir.AluOpType.add,
        replica_groups=[[0, 1], [2, 3], [4, 5], [6, 7]],
        ins=[bin_[:].opt()], outs=[bout[:].opt()],
    )

    # combine: kc = KA*(A1k + (1/L)*N2k/D2) ; vc = VA*(A1v + (1/L)*N2v/D2)
    ar = parts
    nc.sync.dma_start(ar[:], bout[:])
    rd2 = sp.tile([100, 1], F32)
    nc.vector.reciprocal(rd2[:], ar[:, 4096:4097])
    for i, coef in ((0, KA), (1, VA)):
        u = sp.tile([100, 1024], F32, tag="u", bufs=1)
        nc.vector.tensor_scalar_mul(u[:], ar[:, 2048 + i * 1024:3072 + i * 1024], rd2[:])
        us = sp.tile([100, 1024], F32, tag="us", bufs=1)
        nc.scalar.activation(us[:], u[:], AF.Identity, scale=BR)
        w = sp.tile([100, 1024], F32, tag="w", bufs=1)
        nc.vector.tensor_tensor(w[:], us[:], ar[:, i * 1024:(i + 1) * 1024],
                                mybir.AluOpType.add)
        nc.scalar.activation(kcvc[:, i * 1024:(i + 1) * 1024], w[:],
                             AF.Identity, scale=coef)
    for g in range(G):
        ptrb = psa.tile([128, 128], BF16, tag="ps_ab", bufs=1)
        nc.tensor.transpose(ptrb[:, 0:100], kcvc[:, g * 128:(g + 1) * 128],
                            idb[:100, :100])
        nc.vector.tensor_copy(kcT[:, g, :], ptrb[:, 0:100])

    sctx.close()

    # ================= main g-loop =================
    gctx = ExitStack()
    gp = gctx.enter_context(tc.tile_pool(name="gp", bufs=1))
    pg = gctx.enter_context(tc.tile_pool(name="pg", bufs=1, space="PSUM"))

    wq4 = wq_d.rearrange("(c p) q -> c p q", p=128)
    wqb2 = wqb_d.rearrange("(g h p) -> g p h", p=128, g=G)
    wz13 = wz1_d.rearrange("(c p) f -> c p f", p=128)
    wzg12 = wzg1_d.rearrange("(g c p) -> g p c", p=128, g=G)

    for g in range(G):
        wqb_g = gp.tile([128, 16], F32, tag="wqb", bufs=2)
        nc.sync.dma_start(wqb_g[:], wqb2[g])
        g1_g = gp.tile([128, 16], F32, tag="g1", bufs=2)
        nc.sync.dma_start(g1_g[:], wzg12[g])

        wqbf = []
        for c in range(4):
            wf = gp.tile([128, 2048], F32, tag="wq_f", bufs=2)
            nc.sync.dma_start(wf[:], wq4[c][:, g * 2048:(g + 1) * 2048])
            wb = gp.tile([128, 2048], BF16, tag="wq_b", bufs=6)
            nc.vector.tensor_copy(wb[:], wf[:])
            wqbf.append(wb)

        zT = gp.tile([128, 16, T], BF16, tag="zT", bufs=1)
        ssg = pg.tile([1, T], F32, tag="ssg", bufs=1)

        for h in range(16):
            # q^T for (g,h): [128 d, T]
            qT = gp.tile([128, T], BF16, tag="qT", bufs=3)
            for t in range(2):
                pq = pg.tile([128, 512], F32, tag="ps_mm", bufs=2)
                for c in range(4):
                    nc.tensor.matmul(pq[:], wqbf[c][:, h * 128:(h + 1) * 128],
                                     xnT[:, c, t * 512:(t + 1) * 512],
                                     start=(c == 0), stop=(c == 3))
                nc.scalar.activation(qT[:, t * 512:(t + 1) * 512], pq[:],
                                     AF.Identity, bias=wqb_g[:, h:h + 1])
            # attention
            pT = gp.tile([100, T], BF16, tag="pT", bufs=3)
            for t in range(2):
                sl = slice(t * 512, (t + 1) * 512)
                pS = pg.tile([100, 512], F32, tag="ps_mm", bufs=2)
                nc.tensor.matmul(pS[:], kcT[:, g, :], qT[:, sl],
                                 start=True, stop=True)
                nc.scalar.activation(pT[:, sl], pS[:], AF.Exp)
            for t in range(2):
                sl = slice(t * 512, (t + 1) * 512)
                prb = pg.tile([128, 512], F32, tag="prb", bufs=1)
                nc.tensor.matmul(prb[:], ones100[:], pT[:, sl],
                                 start=True, stop=True)
                pz = pg.tile([128, 512], F32, tag="pz", bufs=1)
                nc.tensor.matmul(pz[:], kcvc[:, 1024 + g * 128:1024 + (g + 1) * 128],
                                 pT[:, sl], start=True, stop=True)
                rcp = gp.tile([128, 512], F32, tag="rcp", bufs=2)
                nc.vector.reciprocal(rcp[:], prb[:])
                nc.vector.tensor_tensor(zT[:, h, sl], pz[:], rcp[:],
                                        mybir.AluOpType.mult)
                zsq = gp.tile([128, 512], BF16, tag="zsq", bufs=3)
                nc.scalar.activation(zsq[:], zT[:, h, sl], AF.Square)
                nc.tensor.matmul(ssg[0:1, sl], ones128[:], zsq[:],
                                 start=(h == 0), stop=(h == 15))

        # fold z-sumsq of this group into ssz
        if g == 0:
            nc.vector.tensor_copy(ssz[:], ssg[:])
        else:
            nc.vector.tensor_tensor(ssz[:], ssg[:], ssz[:], mybir.AluOpType.add)

        # h1 partial: h1acc[m,:] += wz_w1[g-rows]^T @ z^T_g  (PSUM-accumulate over k)
        wzbs = []
        for k in range(16):
            wzf = gp.tile([128, FF2], F32, tag="wz_f", bufs=3)
            nc.sync.dma_start(wzf[:], wz13[g * 16 + k])
            wzb = gp.tile([128, FF2], BF16, tag="wz_b", bufs=18)
            nc.vector.tensor_scalar_mul(wzb[:], wzf[:], g1_g[:, k:k + 1])
            wzbs.append(wzb)
        for m in range(8):
            ph = pg.tile([128, FF2], F32, tag="ph1", bufs=1)
            for k in range(16):
                for t in range(2):
                    nc.tensor.matmul(ph[:, t * 512:(t + 1) * 512],
                                     wzbs[k][:, m * 128:(m + 1) * 128],
                                     zT[:, k, t * 512:(t + 1) * 512],
                                     start=(k == 0), stop=(k == 15))
            if g == 0:
                nc.vector.tensor_copy(h1acc[:, m, :], ph[:])
            else:
                nc.vector.tensor_tensor(h1acc[:, m, :], ph[:], h1acc[:, m, :],
                                        mybir.AluOpType.add)

    gctx.close()

    # ================= finalize =================
    fctx = ExitStack()
    fp = fctx.enter_context(tc.tile_pool(name="fp", bufs=1))
    pf = fctx.enter_context(tc.tile_pool(name="pf", bufs=1, space="PSUM"))

    # s = sqrt(GH*DV... ) per-token scale = sqrt(16384/ssz)
    st = fp.tile([1, T], F32)
    nc.scalar.activation(st[:], ssz[:], AF.Sqrt, scale=1.0 / (G * H * DV))
    sr = fp.tile([1, T], F32)
    nc.vector.reciprocal(sr[:], st[:])
    sbc = fp.tile([128, T], F32)
    for t in range(2):
        pb = pf.tile([128, 512], F32, tag="pb", bufs=2)
        nc.tensor.matmul(pb[:], ones_cf[:], sr[:, t * 512:(t + 1) * 512],
                         start=True, stop=True)
        nc.vector.tensor_copy(sbc[:, t * 512:(t + 1) * 512], pb[:])

    # h1 = silu(h1raw * s + b1); accumulate sumsq of h1
    ss2 = pf.tile([1, T], F32, tag="ss2", bufs=1)
    for m in range(8):
        t1 = fp.tile([128, T], F32, tag="t1", bufs=2)
        nc.vector.tensor_tensor(t1[:], h1acc[:, m, :], sbc[:], mybir.AluOpType.mult)
        nc.scalar.activation(h1acc[:, m, :], t1[:], AF.Silu, bias=wzb1[:, m:m + 1])
        h1sq = fp.tile([128, T], BF16, tag="h1sq", bufs=2)
        nc.scalar.activation(h1sq[:], h1acc[:, m, :], AF.Square)
        for t in range(2):
            nc.tensor.matmul(ss2[0:1, t * 512:(t + 1) * 512], ones128[:],
                             h1sq[:, t * 512:(t + 1) * 512],
                             start=(m == 0), stop=(m == 7))

    st2 = fp.tile([1, T], F32)
    nc.scalar.activation(st2[:], ss2[:], AF.Sqrt, scale=1.0 / FF2)
    sr2 = fp.tile([1, T], F32)
    nc.vector.reciprocal(sr2[:], st2[:])
    s2bc = fp.tile([128, T], F32)
    for t in range(2):
        pb = pf.tile([128, 512], F32, tag="pb", bufs=2)
        nc.tensor.matmul(pb[:], ones_cf[:], sr2[:, t * 512:(t + 1) * 512],
                         start=True, stop=True)
        nc.vector.tensor_copy(s2bc[:, t * 512:(t + 1) * 512], pb[:])

    h1n = fp.tile([128, 8, T], BF16)
    for m in range(8):
        nc.vector.tensor_tensor(h1n[:, m, :], h1acc[:, m, :], s2bc[:],
                                mybir.AluOpType.mult)

    # wz_w2 (fold g2 into rows) and final out
    wz23 = wz2_d.rearrange("(c p) f -> c p f", p=128)
    wz2b16 = fp.tile([128, 8, FF], BF16)
    for c in range(8):
        wzf2 = fp.tile([128, FF], F32, tag="wzf2", bufs=2)
        nc.sync.dma_start(wzf2[:], wz23[c])
        nc.vector.tensor_scalar_mul(wz2b16[:, c, :], wzf2[:], g2sb[:, c:c + 1])

    od3 = out_d.rearrange("(n p) f -> n p f", p=128)
    for n in range(TC):
        po = pf.tile([128, FF], F32, tag="po", bufs=2)
        nc.tensor.matmul(po[:], ones_c[:], wzb2_r[:], start=True, stop=False)
        for c in range(8):
            nc.tensor.matmul(po[:], h1n[:, c, n * 128:(n + 1) * 128],
                             wz2b16[:, c, :], start=False, stop=(c == 7))
        ot = fp.tile([128, FF], F32, tag="ot", bufs=2)
        nc.scalar.activation(ot[:], po[:], AF.Silu)
        nc.sync.dma_start(od3[n], ot[:])

    fctx.close()
    ctx.close()


_NC = None


def kernel(**inputs):
    global _NC
    if _NC is None:
        _NC = build()
    xf = np.ascontiguousarray(
        np.asarray(inputs["x"], dtype=np.float32).reshape(B * S, D))
    base = {k: np.ascontiguousarray(np.asarray(v, dtype=np.float32))
            for k, v in inputs.items() if k != "x"}
    in_maps = []
    for c in range(NCORES):
        m = dict(base)
        m["x"] = xf[c * T:(c + 1) * T]
        in_maps.append(m)
    res = bass_utils.run_bass_kernel_spmd(_NC, in_maps, core_ids=list(range(NCORES)))
    outs = [np.asarray(res.results[c]["out"]) for c in range(NCORES)]
    return np.concatenate(outs, axis=0).reshape(B, S, FF).astype(np.float32)


if __name__ == "__main__":
    print("building...")
    nc = build()
    print("built ok")
